# revision 52
# baseline (speedup 1.0000x reference)
"""Bass/Tile TRN2 kernel for EnhancedIPA3 (invariant-point-attention variant).

Sharding: 8 cores = batch(2) x query-block(4).  Each core computes Q/K/V
features for its own 256 rows; K-side features are all-gathered (bf16)
within the 4-core batch group; attention runs sequence-parallel over
query blocks.

v2 schedule: K-side columns are packed first in the fused weight matrix so
the K/V features (the collective payload) are staged as early as possible.
The kf/va all-gathers are split in halves (kfA, kfB, vaA, vaB) so QK
attention starts right after kfA lands while the V-side is still on the
wire.  Gather-in uses coalesced 3D DMAs per head-pair block.

Self-contained: hardcodes all shapes; only depends on numpy + concourse.
"""

import numpy as np
from contextlib import ExitStack

import concourse.bass as bass
import concourse.bacc as bacc
import concourse.mybir as mybir
import concourse.tile as tile
from concourse.bass_utils import run_bass_kernel_spmd
from concourse.masks import make_identity

F32 = mybir.dt.float32
F8 = mybir.dt.float8e4
F32R = mybir.dt.float32r
BF16 = mybir.dt.bfloat16
AF = mybir.ActivationFunctionType
OP = mybir.AluOpType

B, N, CS, H, C, P, V = 2, 1024, 384, 12, 16, 4, 8
EPS = 1e-8
NB = N // 4            # 256 rows per core
NT = NB // 128         # 2 row-tiles per core
KPTS = P + V           # 12 k/v points per head
QPTS = P               # 4 q points per head
# fused weight matrix layout: K-side first (gates the collectives)
K_OFF = 0                      # 192
KP_OFF = 192                   # 6 comps x H*4 = 288; col = j*48 + h*4 + p
VP_OFF = 480                   # 6 comps x H*8 = 576; col = j*96 + h*8 + p
V_OFF = 1056                   # 192
Q_OFF = 1248                   # 192
QP_OFF = 1440                  # 6 comps x H*QPTS = 288; col = j*48 + h*4 + p
G_OFF = 1728                   # 48
WALL_COLS = 1776
CB = [(0, 512), (512, 1024), (1024, 1536), (1536, WALL_COLS)]
FEAT = 64              # per-head feature stride (q/k features and o_all)
FS = 42                # used attention features per head
KROWS = 2 * FS         # 84: rows carried by the kf collective
KSB = FEAT + FS        # 106: SBUF contraction rows (odd head at part. 64)
OCH = 66               # v chans + ones col + pad
FOUT = H * (C + 7 * V)           # 864 output-proj input channels
KCH = 7                # contraction chunks for output proj (last = 98 rows)
GROUPS = [[0, 1, 2, 3], [4, 5, 6, 7]]
NKB = 8                # gathered key blocks of 128
KFC = 6 * NB           # kf cols per core (6 t-blocks x 256 keys)
VAC_H = NT * OCH       # per-head va cols (nt-major within head): 132
VAC = H * VAC_H        # 1584


def _host_prep(inputs):
    """Build the combined/permuted weight matrices and scale tables."""
    import ml_dtypes
    wq, bq = inputs["wq"], inputs["bq"]
    wkv, bkv = inputs["wkv"], inputs["bkv"]
    wqp, bqp = inputs["wqp"], inputs["bqp"]
    wkvp, bkvp = inputs["wkvp"], inputs["bkvp"]
    wg, bg = inputs["wg"], inputs["bg"]
    gw = np.asarray(inputs["geom_weight"], np.float32)
    hw = np.asarray(inputs["head_weights"], np.float32)
    sh = 1.0 / (1.0 + np.exp(-hw))           # sigmoid(head_weights) [H]

    wall = np.zeros((CS + 2, WALL_COLS), np.float32)
    wall[:CS, K_OFF:K_OFF + 192] = wkv[:, :192]
    wall[CS, K_OFF:K_OFF + 192] = bkv[:192]
    wall[:CS, V_OFF:V_OFF + 192] = wkv[:, 192:]
    wall[CS, V_OFF:V_OFF + 192] = bkv[192:]
    wall[:CS, Q_OFF:Q_OFF + 192] = wq
    wall[CS, Q_OFF:Q_OFF + 192] = bq
    wall[:CS, G_OFF:G_OFF + 48] = wg
    wall[CS, G_OFF:G_OFF + 48] = bg
    # k-points then v-points (component-major within each)
    for h in range(H):
        for p in range(KPTS):
            for j in range(6):
                if p < P:
                    d0 = KP_OFF + j * (H * P) + h * P + p
                else:
                    d0 = VP_OFF + j * (H * V) + h * V + (p - P)
                s0 = h * ((P + V) * 6) + p * 6 + j
                wall[:CS, d0] = wkvp[:, s0]
                wall[CS, d0] = bkvp[s0]
    # q points: col = QP_OFF + j*48 + h*4 + p
    for h in range(H):
        for p in range(QPTS):
            for j in range(6):
                d0 = QP_OFF + j * (H * QPTS) + h * QPTS + p
                s0 = h * (P * 6) + p * 6 + j
                wall[:CS, d0] = wqp[:, s0]
                wall[CS, d0] = bqp[s0]
    # two half-bias rows (keeps contraction dims even)
    wall[CS + 1] = wall[CS] * 0.5
    wall[CS] = wall[CS + 1]

    bout_half = np.asarray(inputs["bout"], np.float32)[None, :] * 0.5
    wout_b = np.concatenate(
        [np.asarray(inputs["wout"], np.float32), bout_half, bout_half],
        axis=0)  # [866, 384]

    # per-column scale for the assembled Qfeat [n, H*FEAT]
    qs = np.zeros((FEAT * H,), np.float32)
    for h in range(H):
        o = h * FEAT
        qs[o:o + 16] = sh[h] / np.sqrt(C)        # scalar q . k
        qs[o + 16:o + 28] = sh[h] * gw[0] * 0.5  # 2*gw0/P * (qc.kc), P=4
        qs[o + 28:o + 40] = sh[h] * gw[1]        # gw1 * (qd.kd)
        qs[o + 40] = sh[h]                       # * (-gw0/P * k2sum)
        qs[o + 41] = sh[h]                       # combo col * 1
    qscale = np.broadcast_to(qs, (128, FEAT * H)).copy()

    rot9 = np.ascontiguousarray(
        np.asarray(inputs["rot"], np.float32).reshape(B, N, 9))
    trans = np.asarray(inputs["trans"], np.float32)
    s = np.asarray(inputs["s"], np.float32)
    wall16 = wall.astype(ml_dtypes.bfloat16)
    return s, rot9, trans, wall16, wout_b, qscale, gw


_PROGRAM_CACHE = {}
_DEBUG_DUMP = False


def _build_program(gw0, gw1):
    key = (float(gw0), float(gw1))
    if key in _PROGRAM_CACHE:
        return _PROGRAM_CACHE[key]

    nc = bacc.Bacc("TRN2", target_bir_lowering=False, debug=False, num_devices=8)

    s_loc = nc.dram_tensor("s_loc", [NB, CS], F32, kind="ExternalInput")
    rot_loc = nc.dram_tensor("rot_loc", [NB, 9], F32, kind="ExternalInput")
    trans_loc = nc.dram_tensor("trans_loc", [NB, 3], F32, kind="ExternalInput")
    wall_d = nc.dram_tensor("wall", [CS + 2, WALL_COLS], BF16, kind="ExternalInput")
    wout_d = nc.dram_tensor("wout_b", [FOUT + 2, CS], F32, kind="ExternalInput")
    qscale_d = nc.dram_tensor("qscale", [128, FEAT * H], F32,
                              kind="ExternalInput")
    out_loc = nc.dram_tensor("out_loc", [NB, CS], F32, kind="ExternalOutput")

    kf_loc = nc.dram_tensor("kf_loc", [KROWS, KFC], BF16)
    kf_gath = nc.dram_tensor("kf_gath", [4, KROWS, KFC], BF16)
    va_loc = nc.dram_tensor("va_loc", [128, VAC], F8)
    va_gath = nc.dram_tensor("va_gath", [4, 128, VAC], F8)

    with tile.TileContext(nc) as tc:
        with ExitStack() as ctx:
            _emit(ctx, tc, nc, s_loc, rot_loc, trans_loc, wall_d, wout_d,
                  qscale_d, out_loc,
                  (kf_loc, kf_gath),
                  (va_loc, va_gath), gw0, gw1)

    nc.compile()
    _PROGRAM_CACHE[key] = nc
    return nc


def _emit(ctx, tc, nc, s_loc, rot_loc, trans_loc, wall_d, wout_d, qscale_d,
          out_loc, kf_t, va_t, gw0, gw1):
    PS = bass.MemorySpace.PSUM
    kf_loc, kf_gath = kf_t
    va_loc, va_gath = va_t

    const = ctx.enter_context(tc.tile_pool(name="const", bufs=1))
    work = ctx.enter_context(tc.tile_pool(name="work", bufs=1))
    tmp_pool = ctx.enter_context(tc.tile_pool(name="tmp", bufs=2))
    pA_ctx = ExitStack()
    pA = pA_ctx.enter_context(tc.tile_pool(name="pA", bufs=1))
    pre_ctx = ExitStack()
    tpsum = pre_ctx.enter_context(tc.tile_pool(name="tpsum", bufs=2, space=PS))

    # ---- inputs first (critical path), spread across DMA queues ----------
    s_sb, rot_sb, trans_sb = [], [], []
    for nt in range(NT):
        r = slice(nt * 128, (nt + 1) * 128)
        t = pA.tile([128, CS], F32, name=f"s{nt}")
        nc.sync.dma_start(t[:], s_loc[r, :])
        s_sb.append(t)
        t = const.tile([128, 9], F32, name=f"rot{nt}")
        nc.sync.dma_start(t[:], rot_loc[r, :])
        rot_sb.append(t)
        t = const.tile([128, 3], F32, name=f"trans{nt}")
        nc.sync.dma_start(t[:], trans_loc[r, :])
        trans_sb.append(t)

    wall_sb = [pA.tile([128, WALL_COLS], BF16, name=f"wall{kc}")
               for kc in range(3)]
    wall_bias = pA.tile([2, WALL_COLS], BF16)
    wall_engs = [nc.scalar, nc.sync, nc.sync]
    for kc in range(3):
        wall_engs[kc].dma_start(wall_sb[kc][:],
                                wall_d[kc * 128:(kc + 1) * 128, :])
    nc.scalar.dma_start(wall_bias[:], wall_d[CS:CS + 2, :])

    # ---- constants -------------------------------------------------------
    ident = const.tile([128, 128], F32)
    make_identity(nc, ident[:])
    ident_r = const.tile([128, 128], F32R)
    nc.vector.tensor_copy(ident_r[:], ident[:])
    ident_b = const.tile([128, 128], BF16)
    nc.vector.tensor_copy(ident_b[:], ident[:])
    ones2_f32 = const.tile([2, NB], F32)
    nc.gpsimd.memset(ones2_f32[:], 1.0)
    ones_row = const.tile([2, NB], BF16)
    nc.vector.tensor_copy(ones_row[:], ones2_f32[:])

    # ---- sT (transpose s, cast to bf16) ----------------------------------
    sT = [pA.tile([128, NB], BF16, name=f"sT{kc}") for kc in range(3)]
    for nt in range(NT):
        for kc in range(3):
            ps = tpsum.tile([128, 128], F32, tag="tps")
            nc.tensor.transpose(ps[:], s_sb[nt][:, kc * 128:(kc + 1) * 128], ident[:])
            nc.scalar.copy(sT[kc][:, nt * 128:(nt + 1) * 128], ps[:])

    # ---- projections (K-point cols first: gather-critical) ---------------
    q_sb = [work.tile([128, 192], F32, name=f"q{nt}") for nt in range(NT)]
    k_sb = [work.tile([128, 192], F32, name=f"k{nt}") for nt in range(NT)]
    v_sb = [work.tile([128, 192], F32, name=f"v{nt}") for nt in range(NT)]
    g_sb = [work.tile([128, 48], F32, name=f"g{nt}") for nt in range(NT)]
    kp_sb = [pA.tile([128, 6 * 48], F32, name=f"kp{nt}") for nt in range(NT)]
    vp_sb = [pA.tile([128, 6 * 96], F32, name=f"vp{nt}") for nt in range(NT)]
    qp_sb = [pA.tile([128, 6 * 48], F32, name=f"qp{nt}") for nt in range(NT)]

    regions = [(K_OFF, 192, k_sb, "copy"), (KP_OFF, 288, kp_sb, "relu"),
               (VP_OFF, 576, vp_sb, "relu"),
               (V_OFF, 192, v_sb, "scopy"), (Q_OFF, 192, q_sb, "copy"),
               (QP_OFF, 288, qp_sb, "vrelu"), (G_OFF, 48, g_sb, "sigmoid")]
    ppsum = pre_ctx.enter_context(tc.tile_pool(name="ppsum", bufs=4, space=PS))

    def emit_proj(chunks):
        for (c0, c1), nt in [(cb, nt) for cb in chunks for nt in range(NT)]:
            nsl = slice(nt * 128, (nt + 1) * 128)
            ps = ppsum.tile([128, c1 - c0], F32, tag="proj", name="ps")
            for kc in range(3):
                nc.tensor.matmul(ps[:], sT[kc][:, nsl], wall_sb[kc][:, c0:c1],
                                 start=(kc == 0), stop=False)
            nc.tensor.matmul(ps[:], ones_row[:, nsl], wall_bias[:, c0:c1],
                             start=False, stop=True)
            for (r0, rw, dst, kind) in regions:
                lo, hi = max(r0, c0), min(r0 + rw, c1)
                if lo >= hi:
                    continue
                src = ps[:, lo - c0:hi - c0]
                dv = dst[nt][:, lo - r0:hi - r0]
                if kind == "copy":
                    nc.vector.tensor_copy(dv, src)
                elif kind == "scopy":
                    nc.scalar.copy(dv, src)
                elif kind == "sigmoid":
                    nc.scalar.activation(dv, src, AF.Sigmoid)
                elif kind == "vrelu":
                    # relu on DVE: keeps the Q-projection pipeline off the
                    # clogged scalar queue (PE stalls on PSUM reuse otherwise)
                    nc.vector.tensor_scalar_max(dv, src, 0.0)
                else:
                    nc.scalar.activation(dv, src, AF.Relu)

    # K-point phase: chunk (0,512) covers K + KP (+ head of VP)
    emit_proj([(0, 512)])

    # ---- rigid transform helpers -----------------------------------------
    kpco = [pA.tile([128, 6 * 48], F32, name=f"kpco{nt}") for nt in range(NT)]
    vpco = [pA.tile([128, 6 * 96], F32, name=f"vpco{nt}") for nt in range(NT)]
    qco = [pA.tile([128, 6 * 48], F32, name=f"qco{nt}") for nt in range(NT)]

    def _transform(nt, src, dst, bw):
        """Rigid transform of one comp-major tile: nt0 on vector (AP-scalar
        ops), nt1 dirs on gpsimd (broadcast views) so the two row-tiles
        spread across both engines."""
        rt, tr = rot_sb[nt], trans_sb[nt]

        def pv(j):
            return src[nt][:, j * bw:(j + 1) * bw]
        for i in range(3):
            dco = dst[nt][:, i * bw:(i + 1) * bw]
            ddi = dst[nt][:, (3 + i) * bw:(4 + i) * bw]
            if nt == 0:
                nc.vector.tensor_scalar(dco, pv(0), rt[:, 3 * i:3 * i + 1],
                                        tr[:, i:i + 1], OP.mult, OP.add)
                nc.vector.scalar_tensor_tensor(dco, pv(1),
                                               rt[:, 3 * i + 1:3 * i + 2], dco,
                                               OP.mult, OP.add)
                nc.vector.scalar_tensor_tensor(dco, pv(2),
                                               rt[:, 3 * i + 2:3 * i + 3], dco,
                                               OP.mult, OP.add)
                nc.vector.tensor_scalar_mul(ddi, pv(3), rt[:, 3 * i:3 * i + 1])
                nc.vector.scalar_tensor_tensor(ddi, pv(4),
                                               rt[:, 3 * i + 1:3 * i + 2], ddi,
                                               OP.mult, OP.add)
                nc.vector.scalar_tensor_tensor(ddi, pv(5),
                                               rt[:, 3 * i + 2:3 * i + 3], ddi,
                                               OP.mult, OP.add)
            else:
                tdi = tmp_pool.tile([128, bw], F32, tag=f"tdi{bw}", name="tdi")
                nc.vector.tensor_scalar(dco, pv(0), rt[:, 3 * i:3 * i + 1],
                                        tr[:, i:i + 1], OP.mult, OP.add)
                nc.vector.scalar_tensor_tensor(dco, pv(1),
                                               rt[:, 3 * i + 1:3 * i + 2], dco,
                                               OP.mult, OP.add)
                nc.vector.scalar_tensor_tensor(dco, pv(2),
                                               rt[:, 3 * i + 2:3 * i + 3], dco,
                                               OP.mult, OP.add)
                nc.gpsimd.tensor_tensor(
                    ddi, pv(3), rt[:, 3 * i:3 * i + 1].broadcast_to([128, bw]),
                    OP.mult)
                nc.gpsimd.tensor_tensor(
                    tdi[:], pv(4),
                    rt[:, 3 * i + 1:3 * i + 2].broadcast_to([128, bw]), OP.mult)
                nc.gpsimd.tensor_tensor(ddi, ddi, tdi[:], OP.add)
                nc.gpsimd.tensor_tensor(
                    tdi[:], pv(5),
                    rt[:, 3 * i + 2:3 * i + 3].broadcast_to([128, bw]), OP.mult)
                nc.gpsimd.tensor_tensor(ddi, ddi, tdi[:], OP.add)

    def kpcomp(nt, j):  # [128, H, 4] view of K-point comp j
        blk = kpco[nt][:, j * 48:(j + 1) * 48]
        return blk.rearrange("p (h x) -> p h x", x=P)

    def vpcomp(nt, j):  # [128, H, 8] view of V-point comp j
        blk = vpco[nt][:, j * 96:(j + 1) * 96]
        return blk.rearrange("p (h x) -> p h x", x=V)

    def qcomp(nt, j):  # [128, H, 4] view of Q comp block j
        blk = qco[nt][:, j * 48:(j + 1) * 48]
        return blk.rearrange("p (h x) -> p h x", x=QPTS)

    def q_transform(nt):
        _transform(nt, qp_sb, qco, 48)

    # K-point transform immediately after chunk 1
    for nt in range(NT):
        _transform(nt, kp_sb, kpco, 48)

    # remaining K-side projections (VP tail + V) keep the PE busy meanwhile
    emit_proj([(512, 1024), (1024, 1248)])

    # ---- k2 + Kfeat assembly ---------------------------------------------
    k2c = [work.tile([128, H], F32, name=f"k2c{nt}") for nt in range(NT)]

    def psum4(eng, dst, srct):  # [128,48]=(H,4) -> [128,H]
        sv = srct[:].rearrange("p (h x) -> p h x", x=P)
        eng.tensor_tensor(dst, sv[:, :, 0], sv[:, :, 1], OP.add)
        eng.tensor_tensor(dst, dst, sv[:, :, 2], OP.add)
        eng.tensor_tensor(dst, dst, sv[:, :, 3], OP.add)

    kf = [work.tile([128, FEAT * H], BF16, name=f"kf{nt}") for nt in range(NT)]
    kfstgE = work.tile([128, KFC], BF16, name="kfstgE")
    kfstgO = work.tile([128, KFC], BF16, name="kfstgO")
    vastg = work.tile([128, VAC], F8, name="vastg")
    for nt in range(NT):
        eng = nc.gpsimd if nt else nc.vector
        sq = tmp_pool.tile([128, 48], F32, tag=f"sq{nt}k", name="sq")
        t2 = tmp_pool.tile([128, 48], F32, tag=f"t2{nt}k", name="t2")
        eng.tensor_tensor(sq[:], kpcomp(nt, 0), kpcomp(nt, 0), OP.mult)
        for cc in (1, 2):
            eng.tensor_tensor(t2[:], kpcomp(nt, cc), kpcomp(nt, cc), OP.mult)
            eng.tensor_tensor(sq[:], sq[:], t2[:], OP.add)
        psum4(eng, k2c[nt][:], sq)
        nc.vector.tensor_scalar_mul(k2c[nt][:], k2c[nt][:], -gw0 / P)

        kfv = kf[nt][:].rearrange("p (h f) -> p h f", f=FEAT)
        nc.gpsimd.memset(kfv[:, :, 42:64], 0.0)   # pad read by kf transposes
        nc.vector.tensor_copy(kfv[:, :, 0:16],
                              k_sb[nt][:].rearrange("p (h c) -> p h c", c=16))
        for i in range(3):
            # comp-major blocks: coords at 16+4i, dirs at 28+4i (contiguous)
            nc.vector.tensor_copy(kfv[:, :, 16 + 4 * i:20 + 4 * i],
                                  kpcomp(nt, i))
            nc.scalar.copy(kfv[:, :, 28 + 4 * i:32 + 4 * i],
                           kpcomp(nt, 3 + i))
        nc.gpsimd.tensor_copy(kfv[:, :, 40], k2c[nt][:])
        nc.gpsimd.memset(kfv[:, :, 41], 1.0)

    # ---- kfT transposes (84 staged rows) + DMA + kf collective -----------
    for t in range(6):
        for nt in range(NT):
            ps = tpsum.tile([128, 128], BF16, tag="tpsb")
            nc.tensor.transpose(ps[:], kf[nt][:, t * 128:(t + 1) * 128],
                                ident_b[:])
            c0 = t * NB + nt * 128
            eng = nc.scalar if t % 2 else nc.vector
            if t % 2:
                nc.scalar.copy(kfstgE[0:FS, c0:c0 + 128], ps[0:FS, :])
                nc.scalar.copy(kfstgO[0:FS, c0:c0 + 128], ps[FEAT:FEAT + FS, :])
            else:
                nc.vector.tensor_copy(kfstgE[0:FS, c0:c0 + 128], ps[0:FS, :])
                nc.vector.tensor_copy(kfstgO[0:FS, c0:c0 + 128],
                                      ps[FEAT:FEAT + FS, :])
    nc.sync.dma_start(kf_loc[0:FS, :], kfstgE[0:FS, :])
    nc.sync.dma_start(kf_loc[FS:KROWS, :], kfstgO[0:FS, :])
    # trigger kf right away: nothing else sits ahead of it on gpsimd
    nc.gpsimd.collective_compute(
        "AllGather", OP.bypass, replica_groups=GROUPS,
        ins=[kf_loc[:]], outs=[kf_gath[:]])

    # ---- V-point transform + Vall assembly + va collectives --------------
    for nt in range(NT):
        _transform(nt, vp_sb, vpco, 96)
    for nt in range(NT):
        vav = vastg[:].rearrange("p (h y) -> p h y", y=VAC_H)[
            :, :, nt * OCH:(nt + 1) * OCH]
        nc.scalar.copy(vav[:, :, 0:16],
                       v_sb[nt][:].rearrange("p (h c) -> p h c", c=16))
        for j in range(6):
            eng = nc.vector if nt == 0 else nc.scalar
            if nt == 0:
                nc.vector.tensor_copy(vav[:, :, 16 + j * V:16 + (j + 1) * V],
                                      vpcomp(nt, j))
            else:
                nc.scalar.copy(vav[:, :, 16 + j * V:16 + (j + 1) * V],
                               vpcomp(nt, j))
        nc.gpsimd.memset(vav[:, :, 64], 1.0)
        nc.gpsimd.memset(vav[:, :, 65], 0.0)
    nc.sync.dma_start(va_loc[:, :], vastg[:, :])
    nc.gpsimd.collective_compute(
        "AllGather", OP.bypass, replica_groups=GROUPS,
        ins=[va_loc[:]], outs=[va_gath[:]])

    # ---- Q-phase projections (overlap the collectives) -------------------
    emit_proj([(1248, 1648), (1648, WALL_COLS)])

    # ---- qscale on the scalar queue (needed by qf mid-Q-side) ------------
    qscale_sb = const.tile([128, FEAT * H], F32)
    nc.scalar.dma_start(qscale_sb[:], qscale_d[:, :])

    # ---- Q side (overlaps collectives) -----------------------------------
    # dep-gate: keep gpsimd's in-order queue clear until the kf staging DMA
    # fires so the collective triggers are not stuck behind Q-side gpsimd
    # work (the [0,0] garbage is overwritten by the transform below).
    nc.gpsimd.tensor_copy(qco[1][0:1, 0:1], kfstgO[0:1, 0:1])
    for nt in range(NT):
        q_transform(nt)
        gv = g_sb[nt][:].rearrange("p (h x) -> p h x", x=P)
        for i in range(6):
            eng = nc.vector if i < 3 else nc.gpsimd
            eng.tensor_tensor(qcomp(nt, i), qcomp(nt, i), gv, OP.mult)

    # q2 + curvature: nt0 on vector, nt1 on gpsimd so the two row-tiles'
    # chains run concurrently (this gates qf -> qfT2 -> QK start).
    q2c = [work.tile([128, H], F32, name=f"q2c{nt}") for nt in range(NT)]
    for nt in range(NT):
        eng = nc.vector if nt == 0 else nc.gpsimd
        sq = tmp_pool.tile([128, 48], F32, tag=f"sq{nt}", name="sq")
        t2 = tmp_pool.tile([128, 48], F32, tag=f"t2{nt}", name="t2")
        eng.tensor_tensor(sq[:], qcomp(nt, 0), qcomp(nt, 0), OP.mult)
        for cc in (1, 2):
            eng.tensor_tensor(t2[:], qcomp(nt, cc), qcomp(nt, cc), OP.mult)
            eng.tensor_tensor(sq[:], sq[:], t2[:], OP.add)
        q2s = tmp_pool.tile([128, H], F32, tag=f"q2s{nt}", name="q2s")
        psum4(eng, q2s[:], sq)

        cr = tmp_pool.tile([128, 48], F32, tag=f"cr{nt}", name="cr")
        cs_ = tmp_pool.tile([128, 48], F32, tag=f"cs_{nt}", name="cs_")
        t3 = tmp_pool.tile([128, 48], F32, tag=f"t3{nt}", name="t3")
        first = True
        for (a, b_) in ((1, 2), (2, 0), (0, 1)):
            eng.tensor_tensor(cr[:], qcomp(nt, 3 + a),
                              kpcomp(nt, 3 + b_), OP.mult)
            eng.tensor_tensor(t3[:], qcomp(nt, 3 + b_),
                              kpcomp(nt, 3 + a), OP.mult)
            eng.tensor_tensor(cr[:], cr[:], t3[:], OP.subtract)
            eng.tensor_tensor(cr[:], cr[:], cr[:], OP.mult)
            if first:
                eng.tensor_copy(cs_[:], cr[:])
                first = False
            else:
                eng.tensor_tensor(cs_[:], cs_[:], cr[:], OP.add)
        nq2 = tmp_pool.tile([128, 48], F32, tag=f"nq2{nt}", name="nq2")
        nk2 = tmp_pool.tile([128, 48], F32, tag=f"nk2{nt}", name="nk2")
        eng.tensor_tensor(nq2[:], qcomp(nt, 3), qcomp(nt, 3), OP.mult)
        for cc in (4, 5):
            eng.tensor_tensor(t2[:], qcomp(nt, cc), qcomp(nt, cc), OP.mult)
            eng.tensor_tensor(nq2[:], nq2[:], t2[:], OP.add)
        eng.tensor_tensor(nk2[:], kpcomp(nt, 3), kpcomp(nt, 3),
                          OP.mult)
        for cc in (4, 5):
            eng.tensor_tensor(t2[:], kpcomp(nt, cc),
                              kpcomp(nt, cc), OP.mult)
            eng.tensor_tensor(nk2[:], nk2[:], t2[:], OP.add)
        eng.tensor_tensor(nq2[:], nq2[:], nk2[:], OP.mult)
        nc.scalar.activation(nq2[:], nq2[:], AF.Sqrt)      # |qd||kd|
        eng.tensor_scalar_add(nq2[:], nq2[:], EPS) if nt == 0 else \
            nc.vector.tensor_scalar_add(nq2[:], nq2[:], EPS)
        nc.vector.reciprocal(nq2[:], nq2[:])
        nc.scalar.activation(cs_[:], cs_[:], AF.Sqrt)      # |cross|
        eng.tensor_tensor(cs_[:], cs_[:], nq2[:], OP.mult)
        curv = tmp_pool.tile([128, H], F32, tag=f"curv{nt}", name="curv")
        psum4(eng, curv[:], cs_)
        nc.vector.tensor_scalar_mul(q2c[nt][:], q2s[:], -gw0 / P)
        nc.vector.scalar_tensor_tensor(q2c[nt][:], curv[:], -gw1 / P, q2c[nt][:],
                                       OP.mult, OP.add)

    qf = [work.tile([128, FEAT * H], BF16, name=f"qf{nt}") for nt in range(NT)]
    for nt in range(NT):
        qfv = qf[nt][:].rearrange("p (h f) -> p h f", f=FEAT)
        nc.vector.tensor_copy(qfv[:, :, 0:16],
                              q_sb[nt][:].rearrange("p (h c) -> p h c", c=16))
        for i in range(3):
            nc.vector.tensor_copy(qfv[:, :, 16 + 4 * i:20 + 4 * i],
                                  qcomp(nt, i))
            nc.vector.tensor_copy(qfv[:, :, 28 + 4 * i:32 + 4 * i],
                                  qcomp(nt, 3 + i))
        nc.gpsimd.memset(qfv[:, :, 40], 1.0)
        nc.vector.tensor_copy(qfv[:, :, 41], q2c[nt][:])
        nc.vector.tensor_tensor(qf[nt][:], qf[nt][:], qscale_sb[:], OP.mult)

    # block-diagonal paired-query layout: cols 0:256 even head (rows 0:42),
    # cols 256:512 odd head (rows 42:84); everything else zero.  Engine ops
    # cannot write at partition offset 42, so the odd halves hop through a
    # staging tile and land via SBUF->SBUF DMA (no offset restriction).
    qfT2 = [work.tile([KROWS, 2 * NB], BF16, name=f"qfT{t}") for t in range(6)]
    qto = work.tile([128, 6 * NB], BF16, name="qto")
    for t in range(6):
        nc.gpsimd.memset(qfT2[t][:], 0.0)
        for nt in range(NT):
            ps = tpsum.tile([128, 128], BF16, tag="tpsb")
            nc.tensor.transpose(ps[:], qf[nt][:, t * 128:(t + 1) * 128],
                                ident_b[:])
            eng = nc.scalar if t % 2 else nc.vector
            if t % 2:
                nc.scalar.copy(qfT2[t][0:FS, nt * 128:(nt + 1) * 128],
                               ps[0:FS, :])
                nc.scalar.copy(qto[0:FS, t * NB + nt * 128:t * NB + (nt + 1) * 128],
                               ps[FEAT:FEAT + FS, :])
            else:
                nc.vector.tensor_copy(qfT2[t][0:FS, nt * 128:(nt + 1) * 128],
                                      ps[0:FS, :])
                nc.vector.tensor_copy(
                    qto[0:FS, t * NB + nt * 128:t * NB + (nt + 1) * 128],
                    ps[FEAT:FEAT + FS, :])
    for t in range(6):
        nc.scalar.dma_start(qfT2[t][FS:KROWS, NB:2 * NB],
                            qto[0:FS, t * NB:(t + 1) * NB])

    # ---- gather-in (coalesced 3D DMAs, pipelined with attention) ---------
    pA_ctx.close()
    attA = ctx.enter_context(tc.tile_pool(name="attA", bufs=1))
    # kfAllT col = t*1024 + g*256 + half*128; rows 0:42 even head, 42:84 odd
    # (DMA writes have no partition-offset restriction)
    kfAllT = attA.tile([KROWS, 6 * 1024], BF16, name="kfAllT")
    # vaAll per-g block [128, VAC], head-major inside; the fp8 wire copy
    # lands in vaAll8 and is upcast to bf16 for the AV matmuls
    vaAll = attA.tile([128, 4 * VAC], BF16, name="vaAll")
    vaAll8 = attA.tile([128, 4 * VAC], F8, name="vaAll8")
    for t in range(6):
        dst = kfAllT[:, t * 1024:(t + 1) * 1024].rearrange(
            "p (g c) -> p g c", g=4)
        nc.sync.dma_start(dst, kf_gath[:, :, t * NB:(t + 1) * NB].rearrange(
            "g p c -> p g c"))
    dstva = vaAll8[:].rearrange("p (g c) -> p g c", c=VAC)
    nc.sync.dma_start(dstva, va_gath[:, :, :].rearrange("g p c -> p g c"))
    for g in range(4):
        cv = slice(g * VAC, (g + 1) * VAC)
        nc.vector.tensor_copy(vaAll[:, cv], vaAll8[:, cv])



    # ---- attention -------------------------------------------------------
    pre_ctx.close()
    att_ctx = ExitStack()
    apsum = att_ctx.enter_context(tc.tile_pool(name="apsum", bufs=2, space=PS))
    opsum = att_ctx.enter_context(tc.tile_pool(name="opsum", bufs=2, space=PS))
    # expT layout per head-pair t: col = kb*512 + parity*256 + q
    expT_tiles = [attA.tile([128, 4096], BF16, name=f"expT{i}")
                  for i in range(6)]
    o_all = [work.tile([128, FEAT * H], F32, name=f"oall{qt}") for qt in range(NT)]
    sums = [work.tile([128, H], F32, name=f"sums{qt}") for qt in range(NT)]

    # all QK+exp first (only needs kf); AV chases the va collectives
    for t in range(6):
        expT = expT_tiles[t]
        for pair in range(4):
            aps = apsum.tile([128, 1024], F32, tag="attT", name="aps")
            for half in range(2):
                kb = 2 * pair + half
                nc.tensor.matmul(
                    aps[:, half * 512:(half + 1) * 512],
                    kfAllT[:, t * 1024 + kb * 128:t * 1024 + (kb + 1) * 128],
                    qfT2[t][:, :],
                    start=True, stop=True)
            nc.scalar.activation(
                expT[:, pair * 1024:(pair + 1) * 1024], aps[:], AF.Exp)

    # deferred output-proj weights.  The 1-elem copies from qfT2 are
    # artificial dependencies: without them the scheduler starts these
    # 1.3MB loads (and the fp32r casts) at t=0, stealing front DMA/gpsimd
    # time from the collective-staging critical path.  Plain F32 DMA: an
    # F32R-dtyped DMA poisons concurrent bf16 transfers on shared SDMA
    # engines (tf32 rounding of word pairs); the fp32r rounding happens in
    # an on-chip copy that rides gpsimd while the QK/exp stream runs.
    wout_f, wout_sb = [], []
    for kc in range(KCH):
        r0 = kc * 128
        r1 = min(FOUT + 2, r0 + 128)
        t = const.tile([r1 - r0, CS], F32, name=f"woutf{kc}")
        nc.gpsimd.tensor_copy(t[0:1, 0:1], qfT2[0][0:1, 0:1])
        nc.sync.dma_start(t[:], wout_d[r0:r1, :])
        wout_f.append(t)
        wout_sb.append(const.tile([r1 - r0, CS], F32R, name=f"wout{kc}"))
    for kc in range(KCH):
        nc.gpsimd.tensor_copy(wout_sb[kc][:], wout_f[kc][:])

    # ---- inverse transform helper (per qt, per 6-head half) --------------
    feats = [work.tile([128, FOUT], F32, name=f"feats{qt}") for qt in range(NT)]

    def emit_inverse(qt, h0, nh):
        # qt 0 runs on vector (AP-scalar ops), qt 1 on gpsimd (broadcast
        # views) so the two row-tiles proceed concurrently.
        hs = slice(h0, h0 + nh)
        W = nh * V
        ovv = o_all[qt][:].rearrange("p (h f) -> p h f", f=FEAT)[:, hs]

        def og(j):  # [128, nh, V] component j (component-major o layout)
            return ovv[:, :, 16 + j * V:16 + (j + 1) * V]

        gview = feats[qt][:, 192:FOUT].rearrange(
            "p (h x c) -> p h x c", h=H, c=7)[:, hs]

        def gcol(i):  # [128, nh*V] flat strided view of feats column i
            return gview[:, :, :, i].rearrange("p h x -> p (h x)")

        rt, tr = rot_sb[qt], trans_sb[qt]
        shp = [128, nh, V]
        eng = nc.vector if qt == 0 else nc.gpsimd

        # fold the softmax normalization in here: o_scalar and coord points
        # are divided by the exp-sum; direction points skip it (they get
        # re-normalized anyway).
        rec6 = tmp_pool.tile([128, nh], F32, tag=f"rec6{qt}{nh}", name="rec6")
        nc.vector.reciprocal(rec6[:], sums[qt][:, hs])
        fview = feats[qt][:, h0 * 16:(h0 + nh) * 16].rearrange(
            "p (h c) -> p h c", c=16)
        eng.tensor_tensor(fview, ovv[:, :, 0:16],
                          rec6[:].unsqueeze(2).broadcast_to([128, nh, 16]),
                          OP.mult)

        ogs = [tmp_pool.tile([128, W], F32, tag=f"ogs{j}{qt}{nh}",
                             name=f"ogs{j}") for j in range(3)]
        ld = [tmp_pool.tile([128, W], F32, tag=f"ld{i}{qt}{nh}",
                            name=f"ld{i}") for i in range(3)]
        n2 = tmp_pool.tile([128, W], F32, tag=f"n2{qt}{nh}", name="n2")
        t2b = tmp_pool.tile([128, W], F32, tag=f"t2b{qt}{nh}", name="t2b")

        def v3(t_):
            return t_[:].rearrange("p (h x) -> p h x", x=V)

        r6b = rec6[:].unsqueeze(2).broadcast_to(shp)
        for j in range(3):
            eng.tensor_tensor(v3(ogs[j]), og(j), r6b, OP.mult)
        if qt == 0:
            for j in range(3):
                eng.tensor_scalar(ogs[j][:], ogs[j][:], tr[:, j:j + 1], None,
                                  OP.subtract)
            for i in range(3):
                # lc accumulates directly into the feats column
                eng.tensor_scalar_mul(gcol(i), ogs[0][:], rt[:, i:i + 1])
                eng.scalar_tensor_tensor(gcol(i), ogs[1][:], rt[:, 3 + i:4 + i],
                                         gcol(i), OP.mult, OP.add)
                eng.scalar_tensor_tensor(gcol(i), ogs[2][:], rt[:, 6 + i:7 + i],
                                         gcol(i), OP.mult, OP.add)
                eng.tensor_scalar_mul(v3(ld[i]), og(3), rt[:, i:i + 1])
                eng.scalar_tensor_tensor(v3(ld[i]), og(4), rt[:, 3 + i:4 + i],
                                         v3(ld[i]), OP.mult, OP.add)
                eng.scalar_tensor_tensor(v3(ld[i]), og(5), rt[:, 6 + i:7 + i],
                                         v3(ld[i]), OP.mult, OP.add)
        else:
            # lc path stays on gpsimd; ld path goes to vector (AP-scalar ops)
            # so the two row-tiles' tail work spreads across both engines.
            tmpg = tmp_pool.tile([128, W], F32, tag=f"tmpg{qt}{nh}",
                                 name="tmpg")
            for j in range(3):
                eng.tensor_tensor(ogs[j][:], ogs[j][:],
                                  tr[:, j:j + 1].broadcast_to([128, W]),
                                  OP.subtract)
            for i in range(3):
                gc = gview[:, :, :, i]
                eng.tensor_tensor(gc, v3(ogs[0]),
                                  rt[:, i:i + 1].broadcast_to(shp), OP.mult)
                eng.tensor_tensor(v3(tmpg), v3(ogs[1]),
                                  rt[:, 3 + i:4 + i].broadcast_to(shp), OP.mult)
                eng.tensor_tensor(gc, gc, v3(tmpg), OP.add)
                eng.tensor_tensor(v3(tmpg), v3(ogs[2]),
                                  rt[:, 6 + i:7 + i].broadcast_to(shp), OP.mult)
                eng.tensor_tensor(gc, gc, v3(tmpg), OP.add)
                nc.vector.tensor_scalar_mul(v3(ld[i]), og(3), rt[:, i:i + 1])
                nc.vector.scalar_tensor_tensor(v3(ld[i]), og(4),
                                               rt[:, 3 + i:4 + i], v3(ld[i]),
                                               OP.mult, OP.add)
                nc.vector.scalar_tensor_tensor(v3(ld[i]), og(5),
                                               rt[:, 6 + i:7 + i], v3(ld[i]),
                                               OP.mult, OP.add)

        dng = nc.gpsimd if qt == 0 else nc.vector   # ld-norm engine (opposite)
        eng.tensor_tensor(n2[:], gcol(0), gcol(0), OP.mult)
        for i in (1, 2):
            eng.tensor_tensor(t2b[:], gcol(i), gcol(i), OP.mult)
            eng.tensor_tensor(n2[:], n2[:], t2b[:], OP.add)
        nc.scalar.activation(gcol(6), n2[:], AF.Sqrt)
        n2d = tmp_pool.tile([128, W], F32, tag=f"n2d{qt}{nh}", name="n2d")
        t2d = tmp_pool.tile([128, W], F32, tag=f"t2d{qt}{nh}", name="t2d")
        dng.tensor_tensor(n2d[:], ld[0][:], ld[0][:], OP.mult)
        for i in (1, 2):
            dng.tensor_tensor(t2d[:], ld[i][:], ld[i][:], OP.mult)
            dng.tensor_tensor(n2d[:], n2d[:], t2d[:], OP.add)
        nc.scalar.activation(n2d[:], n2d[:], AF.Sqrt)
        nc.vector.tensor_scalar_max(n2d[:], n2d[:], EPS)
        nc.vector.reciprocal(n2d[:], n2d[:])
        for i in range(3):
            dng.tensor_tensor(gcol(3 + i), ld[i][:], n2d[:], OP.mult)

    def emit_av_mm(h):
        expT = expT_tiles[h // 2]
        par = (h % 2) * NB
        ot_ps = opsum.tile([OCH, NB], F32, tag="otacc", name="ot_ps")
        for kb in range(NKB):
            voff = (kb // 2) * VAC + h * VAC_H + (kb % 2) * OCH
            ecol = kb * 512 + par
            nc.tensor.matmul(
                ot_ps[:],
                vaAll[:, voff:voff + OCH],
                expT[:, ecol:ecol + NB],
                start=(kb == 0), stop=(kb == NKB - 1))
        # early drains stay off the scalar queue so the exp stream never
        # waits behind them; late heads drain via scalar (exps done) so the
        # vector queue is free for the interleaved inverse.
        ot_sb = tmp_pool.tile([OCH, NB], F32R, tag="otsb", name="otsb", bufs=3)
        if h >= 8:
            nc.scalar.copy(ot_sb[:], ot_ps[:])
        else:
            nc.vector.tensor_copy(ot_sb[:], ot_ps[:])
        return ot_sb

    def emit_av_out(h, ot_sb):
        for qt in range(NT):
            tp = opsum.tile([128, OCH], F32R, tag="otp", name="tp")
            nc.tensor.transpose(tp[:], ot_sb[:, qt * 128:(qt + 1) * 128],
                                ident_r[0:OCH, 0:OCH])
            # unnormalized output + exp-sum; division happens in the inverse.
            # qt0 lands via vector, qt1 via scalar to balance the two queues.
            if qt == 0:
                nc.vector.tensor_copy(o_all[qt][:, h * FEAT:h * FEAT + 64],
                                      tp[:, 0:64].bitcast(F32))
                nc.vector.tensor_copy(sums[qt][:, h:h + 1],
                                      tp[:, 64:65].bitcast(F32))
            else:
                nc.scalar.copy(o_all[qt][:, h * FEAT:h * FEAT + 64],
                               tp[:, 0:64].bitcast(F32))
                nc.scalar.copy(sums[qt][:, h:h + 1], tp[:, 64:65].bitcast(F32))

    # AV output transposes are deferred by one head so the PE never waits on
    # the vector drain of ot_sb; the inverse for heads 0-5 fires as soon as
    # head 5's outputs land.
    pend = []
    for h in range(H):
        while pend:
            hp, sb = pend.pop(0)
            emit_av_out(hp, sb)
            if hp == 5:
                for qt in range(NT):
                    emit_inverse(qt, 0, 6)
            elif hp == 9:
                for qt in range(NT):
                    emit_inverse(qt, 6, 4)
        pend.append((h, emit_av_mm(h)))
    while pend:
        hp, sb = pend.pop(0)
        emit_av_out(hp, sb)

    # ---- tail: last head-pair inverse + out-proj -------------------------
    att_ctx.close()
    tpsum2 = ctx.enter_context(tc.tile_pool(name="tpsum2", bufs=2, space=PS))
    opsum2 = ctx.enter_context(tc.tile_pool(name="opsum2", bufs=2, space=PS))
    for qt in range(NT):
        emit_inverse(qt, 10, 2)

    fT = []
    for kc in range(KCH):
        r0 = kc * 128
        rw = min(FOUT, r0 + 128) - r0          # 128 or 96
        pw = rw + 2 if kc == KCH - 1 else rw   # +2 ones rows on last chunk
        t = work.tile([pw, NB], F32R, name=f"fT{kc}")
        fT.append(t)
    lastr = FOUT - (KCH - 1) * 128
    nc.vector.tensor_copy(fT[KCH - 1][lastr:lastr + 2, :], ones2_f32[:])
    ps_out = [opsum2.tile([128, CS], F32, tag=f"oproj{qt}", name="ps")
              for qt in range(NT)]
    # kc 0..3 read only hh=0 regions of feats (+ the o_scalar copies done
    # first thing in emit_inverse) so their transposes/matmuls run on PE
    # while the hh=1 inverse is still on vector/gpsimd.
    for kc in range(KCH):
        r0 = kc * 128
        rw = min(FOUT, r0 + 128) - r0
        for qt in range(NT):
            ps = tpsum2.tile([128, 128], F32, tag="tps2")
            nc.tensor.transpose(ps[:rw, :], feats[qt][:, r0:r0 + rw], ident[:])
            if (kc + qt) % 2:
                nc.vector.tensor_copy(fT[kc][:rw, qt * 128:(qt + 1) * 128],
                                      ps[:rw, :])
            else:
                nc.scalar.copy(fT[kc][:rw, qt * 128:(qt + 1) * 128], ps[:rw, :])
        for qt in range(NT):
            nc.tensor.matmul(ps_out[qt][:], fT[kc][:, qt * 128:(qt + 1) * 128],
                             wout_sb[kc][:], start=(kc == 0),
                             stop=(kc == KCH - 1))
    for qt in range(NT):
        osb = tmp_pool.tile([128, CS], F32, tag="osb", name="osb")
        nc.scalar.copy(osb[:], ps_out[qt][:])
        nc.sync.dma_start(out_loc[qt * 128:(qt + 1) * 128, :], osb[:])

    if _DEBUG_DUMP:
        dbg_kfg = nc.dram_tensor("dbg_kfg", [4, KROWS, KFC], BF16,
                                 kind="ExternalOutput")

        dbg_oall = nc.dram_tensor("dbg_oall", [NT, 128, FEAT * H], F32,
                                  kind="ExternalOutput")
        dbg_sums = nc.dram_tensor("dbg_sums", [NT, 128, H], F32,
                                  kind="ExternalOutput")
        nc.sync.dma_start(dbg_kfg[:], kf_gath[:])
        pass  # debug va dump removed
        for qt in range(NT):
            nc.sync.dma_start(dbg_oall[qt], o_all[qt][:])
            nc.sync.dma_start(dbg_sums[qt], sums[qt][:])


def _run(inputs, trace=False):
    s, rot9, trans, wall, wout_b, qscale, gw = _host_prep(inputs)
    nc = _build_program(float(gw[0]), float(gw[1]))
    in_maps = []
    for c in range(8):
        b, qb = c // 4, c % 4
        r = slice(qb * NB, (qb + 1) * NB)
        in_maps.append({
            "s_loc": np.ascontiguousarray(s[b, r]),
            "rot_loc": np.ascontiguousarray(rot9[b, r]),
            "trans_loc": np.ascontiguousarray(trans[b, r]),
            "wall": wall, "wout_b": wout_b, "qscale": qscale,
        })
    res = run_bass_kernel_spmd(nc, in_maps, list(range(8)), trace=trace)
    out = np.empty((B, N, CS), np.float32)
    for c in range(8):
        b, qb = c // 4, c % 4
        out[b, qb * NB:(qb + 1) * NB] = res.results[c]["out_loc"]
    return out, res


_LAST_RES = None


def kernel(**inputs):
    global _LAST_RES
    out, _LAST_RES = _run(inputs, trace=False)
    return out


def kernel_traced(**inputs):
    return _run(inputs, trace=True)


# revision 53
# speedup vs baseline: 1.0883x; 1.0883x over previous
"""Bass/Tile TRN2 kernel for EnhancedIPA3 (invariant-point-attention variant).

Sharding: 8 cores = batch(2) x query-block(4).  Each core computes Q/K/V
features for its own 256 rows; K-side features are all-gathered (bf16)
within the 4-core batch group; attention runs sequence-parallel over
query blocks.

v2 schedule: K-side columns are packed first in the fused weight matrix so
the K/V features (the collective payload) are staged as early as possible.
The kf/va all-gathers are split in halves (kfA, kfB, vaA, vaB) so QK
attention starts right after kfA lands while the V-side is still on the
wire.  Gather-in uses coalesced 3D DMAs per head-pair block.

Self-contained: hardcodes all shapes; only depends on numpy + concourse.
"""

import numpy as np
from contextlib import ExitStack

import concourse.bass as bass
import concourse.bacc as bacc
import concourse.mybir as mybir
import concourse.tile as tile
from concourse.bass_utils import run_bass_kernel_spmd
from concourse.masks import make_identity

F32 = mybir.dt.float32
F8 = mybir.dt.float8e4
F32R = mybir.dt.float32r
BF16 = mybir.dt.bfloat16
AF = mybir.ActivationFunctionType
OP = mybir.AluOpType

B, N, CS, H, C, P, V = 2, 1024, 384, 12, 16, 4, 8
EPS = 1e-8
NB = N // 4            # 256 rows per core
NT = NB // 128         # 2 row-tiles per core
KPTS = P + V           # 12 k/v points per head
QPTS = P               # 4 q points per head
# fused weight matrix layout: K-side first (gates the collectives)
K_OFF = 0                      # 192
KP_OFF = 192                   # 6 comps x H*4 = 288; col = j*48 + h*4 + p
VP_OFF = 480                   # 6 comps x H*8 = 576; col = j*96 + h*8 + p
V_OFF = 1056                   # 192
Q_OFF = 1248                   # 192
QP_OFF = 1440                  # 6 comps x H*QPTS = 288; col = j*48 + h*4 + p
G_OFF = 1728                   # 48
WALL_COLS = 1776
CB = [(0, 512), (512, 1024), (1024, 1536), (1536, WALL_COLS)]
FEAT = 64              # per-head feature stride (q/k features and o_all)
FS = 42                # used attention features per head
KROWS = 2 * FS         # 84: rows carried by the kf collective
KSB = FEAT + FS        # 106: SBUF contraction rows (odd head at part. 64)
OCH = 66               # v chans + ones col + pad
FOUT = H * (C + 7 * V)           # 864 output-proj input channels
KCH = 7                # contraction chunks for output proj (last = 98 rows)
GROUPS = [[0, 1, 2, 3], [4, 5, 6, 7]]
NKB = 8                # gathered key blocks of 128
KFC = 6 * NB           # kf cols per core (6 t-blocks x 256 keys)
VAC_H = NT * OCH       # per-head va cols (nt-major within head): 132
VAC = H * VAC_H        # 1584


def _host_prep(inputs):
    """Build the combined/permuted weight matrices and scale tables."""
    import ml_dtypes
    wq, bq = inputs["wq"], inputs["bq"]
    wkv, bkv = inputs["wkv"], inputs["bkv"]
    wqp, bqp = inputs["wqp"], inputs["bqp"]
    wkvp, bkvp = inputs["wkvp"], inputs["bkvp"]
    wg, bg = inputs["wg"], inputs["bg"]
    gw = np.asarray(inputs["geom_weight"], np.float32)
    hw = np.asarray(inputs["head_weights"], np.float32)
    sh = 1.0 / (1.0 + np.exp(-hw))           # sigmoid(head_weights) [H]

    wall = np.zeros((CS + 2, WALL_COLS), np.float32)
    wall[:CS, K_OFF:K_OFF + 192] = wkv[:, :192]
    wall[CS, K_OFF:K_OFF + 192] = bkv[:192]
    wall[:CS, V_OFF:V_OFF + 192] = wkv[:, 192:]
    wall[CS, V_OFF:V_OFF + 192] = bkv[192:]
    wall[:CS, Q_OFF:Q_OFF + 192] = wq
    wall[CS, Q_OFF:Q_OFF + 192] = bq
    wall[:CS, G_OFF:G_OFF + 48] = wg
    wall[CS, G_OFF:G_OFF + 48] = bg
    # k-points then v-points (component-major within each)
    for h in range(H):
        for p in range(KPTS):
            for j in range(6):
                if p < P:
                    d0 = KP_OFF + j * (H * P) + h * P + p
                else:
                    d0 = VP_OFF + j * (H * V) + h * V + (p - P)
                s0 = h * ((P + V) * 6) + p * 6 + j
                wall[:CS, d0] = wkvp[:, s0]
                wall[CS, d0] = bkvp[s0]
    # q points: col = QP_OFF + j*48 + h*4 + p
    for h in range(H):
        for p in range(QPTS):
            for j in range(6):
                d0 = QP_OFF + j * (H * QPTS) + h * QPTS + p
                s0 = h * (P * 6) + p * 6 + j
                wall[:CS, d0] = wqp[:, s0]
                wall[CS, d0] = bqp[s0]
    # two half-bias rows (keeps contraction dims even)
    wall[CS + 1] = wall[CS] * 0.5
    wall[CS] = wall[CS + 1]

    bout_half = np.asarray(inputs["bout"], np.float32)[None, :] * 0.5
    wout_b = np.concatenate(
        [np.asarray(inputs["wout"], np.float32), bout_half, bout_half],
        axis=0)  # [866, 384]

    # per-column scale for the assembled Qfeat [n, H*FEAT]
    qs = np.zeros((FEAT * H,), np.float32)
    for h in range(H):
        o = h * FEAT
        qs[o:o + 16] = sh[h] / np.sqrt(C)        # scalar q . k
        qs[o + 16:o + 28] = sh[h] * gw[0] * 0.5  # 2*gw0/P * (qc.kc), P=4
        qs[o + 28:o + 40] = sh[h] * gw[1]        # gw1 * (qd.kd)
        qs[o + 40] = sh[h]                       # * (-gw0/P * k2sum)
        qs[o + 41] = sh[h]                       # combo col * 1
    qscale = np.broadcast_to(qs, (128, FEAT * H)).copy()

    rot9 = np.ascontiguousarray(
        np.asarray(inputs["rot"], np.float32).reshape(B, N, 9))
    trans = np.asarray(inputs["trans"], np.float32)
    s = np.asarray(inputs["s"], np.float32)
    wall16 = wall.astype(ml_dtypes.bfloat16)
    return s, rot9, trans, wall16, wout_b, qscale, gw


_PROGRAM_CACHE = {}
_DEBUG_DUMP = False


def _build_program(gw0, gw1):
    key = (float(gw0), float(gw1))
    if key in _PROGRAM_CACHE:
        return _PROGRAM_CACHE[key]

    nc = bacc.Bacc("TRN2", target_bir_lowering=False, debug=False, num_devices=8)

    s_loc = nc.dram_tensor("s_loc", [NB, CS], F32, kind="ExternalInput")
    rot_loc = nc.dram_tensor("rot_loc", [NB, 9], F32, kind="ExternalInput")
    trans_loc = nc.dram_tensor("trans_loc", [NB, 3], F32, kind="ExternalInput")
    wall_d = nc.dram_tensor("wall", [CS + 2, WALL_COLS], BF16, kind="ExternalInput")
    wout_d = nc.dram_tensor("wout_b", [FOUT + 2, CS], F32, kind="ExternalInput")
    qscale_d = nc.dram_tensor("qscale", [128, FEAT * H], F32,
                              kind="ExternalInput")
    out_loc = nc.dram_tensor("out_loc", [NB, CS], F32, kind="ExternalOutput")

    kf_loc = nc.dram_tensor("kf_loc", [KROWS, KFC], BF16)
    kf_gath = nc.dram_tensor("kf_gath", [4, KROWS, KFC], BF16)
    va_loc = nc.dram_tensor("va_loc", [128, VAC], F8)
    va_gath = nc.dram_tensor("va_gath", [4, 128, VAC], F8)

    with tile.TileContext(nc) as tc:
        with ExitStack() as ctx:
            _emit(ctx, tc, nc, s_loc, rot_loc, trans_loc, wall_d, wout_d,
                  qscale_d, out_loc,
                  (kf_loc, kf_gath),
                  (va_loc, va_gath), gw0, gw1)

    nc.compile()
    _PROGRAM_CACHE[key] = nc
    return nc


def _emit(ctx, tc, nc, s_loc, rot_loc, trans_loc, wall_d, wout_d, qscale_d,
          out_loc, kf_t, va_t, gw0, gw1):
    PS = bass.MemorySpace.PSUM
    kf_loc, kf_gath = kf_t
    va_loc, va_gath = va_t

    const = ctx.enter_context(tc.tile_pool(name="const", bufs=1))
    work = ctx.enter_context(tc.tile_pool(name="work", bufs=1))
    tmp_pool = ctx.enter_context(tc.tile_pool(name="tmp", bufs=2))
    pA_ctx = ExitStack()
    pA = pA_ctx.enter_context(tc.tile_pool(name="pA", bufs=1))
    pre_ctx = ExitStack()
    tpsum = pre_ctx.enter_context(tc.tile_pool(name="tpsum", bufs=2, space=PS))

    # ---- inputs first (critical path), spread across DMA queues ----------
    s_sb, rot_sb, trans_sb = [], [], []
    for nt in range(NT):
        r = slice(nt * 128, (nt + 1) * 128)
        t = pA.tile([128, CS], F32, name=f"s{nt}")
        nc.sync.dma_start(t[:], s_loc[r, :])
        s_sb.append(t)
        t = const.tile([128, 9], F32, name=f"rot{nt}")
        nc.sync.dma_start(t[:], rot_loc[r, :])
        rot_sb.append(t)
        t = const.tile([128, 3], F32, name=f"trans{nt}")
        nc.sync.dma_start(t[:], trans_loc[r, :])
        trans_sb.append(t)

    wall_sb = [pA.tile([128, WALL_COLS], BF16, name=f"wall{kc}")
               for kc in range(3)]
    wall_bias = pA.tile([2, WALL_COLS], BF16)
    wall_engs = [nc.scalar, nc.sync, nc.sync]
    for kc in range(3):
        wall_engs[kc].dma_start(wall_sb[kc][:],
                                wall_d[kc * 128:(kc + 1) * 128, :])
    nc.scalar.dma_start(wall_bias[:], wall_d[CS:CS + 2, :])

    # ---- constants -------------------------------------------------------
    ident = const.tile([128, 128], F32)
    make_identity(nc, ident[:])
    ident_r = const.tile([128, 128], F32R)
    nc.vector.tensor_copy(ident_r[:], ident[:])
    ident_b = const.tile([128, 128], BF16)
    nc.vector.tensor_copy(ident_b[:], ident[:])
    ones2_f32 = const.tile([2, NB], F32)
    nc.gpsimd.memset(ones2_f32[:], 1.0)
    ones_row = const.tile([2, NB], BF16)
    nc.vector.tensor_copy(ones_row[:], ones2_f32[:])

    # ---- sT (transpose s, cast to bf16) ----------------------------------
    sT = [pA.tile([128, NB], BF16, name=f"sT{kc}") for kc in range(3)]
    for nt in range(NT):
        for kc in range(3):
            ps = tpsum.tile([128, 128], F32, tag="tps")
            nc.tensor.transpose(ps[:], s_sb[nt][:, kc * 128:(kc + 1) * 128], ident[:])
            nc.scalar.copy(sT[kc][:, nt * 128:(nt + 1) * 128], ps[:])

    # ---- projections (K-point cols first: gather-critical) ---------------
    q_sb = [work.tile([128, 192], F32, name=f"q{nt}") for nt in range(NT)]
    k_sb = [work.tile([128, 192], F32, name=f"k{nt}") for nt in range(NT)]
    v_sb = [work.tile([128, 192], F32, name=f"v{nt}") for nt in range(NT)]
    g_sb = [work.tile([128, 48], F32, name=f"g{nt}") for nt in range(NT)]
    kp_sb = [pA.tile([128, 6 * 48], F32, name=f"kp{nt}") for nt in range(NT)]
    vp_sb = [pA.tile([128, 6 * 96], F32, name=f"vp{nt}") for nt in range(NT)]
    qp_sb = [pA.tile([128, 6 * 48], F32, name=f"qp{nt}") for nt in range(NT)]

    regions = [(K_OFF, 192, k_sb, "copy"), (KP_OFF, 288, kp_sb, "relu"),
               (VP_OFF, 576, vp_sb, "relu"),
               (V_OFF, 192, v_sb, "scopy"), (Q_OFF, 192, q_sb, "copy"),
               (QP_OFF, 288, qp_sb, "vrelu"), (G_OFF, 48, g_sb, "sigmoid")]
    ppsum = pre_ctx.enter_context(tc.tile_pool(name="ppsum", bufs=4, space=PS))

    def emit_proj(chunks):
        for (c0, c1), nt in [(cb, nt) for cb in chunks for nt in range(NT)]:
            nsl = slice(nt * 128, (nt + 1) * 128)
            ps = ppsum.tile([128, c1 - c0], F32, tag="proj", name="ps")
            for kc in range(3):
                nc.tensor.matmul(ps[:], sT[kc][:, nsl], wall_sb[kc][:, c0:c1],
                                 start=(kc == 0), stop=False)
            nc.tensor.matmul(ps[:], ones_row[:, nsl], wall_bias[:, c0:c1],
                             start=False, stop=True)
            for (r0, rw, dst, kind) in regions:
                lo, hi = max(r0, c0), min(r0 + rw, c1)
                if lo >= hi:
                    continue
                src = ps[:, lo - c0:hi - c0]
                dv = dst[nt][:, lo - r0:hi - r0]
                if kind == "copy":
                    nc.vector.tensor_copy(dv, src)
                elif kind == "scopy":
                    nc.scalar.copy(dv, src)
                elif kind == "sigmoid":
                    nc.scalar.activation(dv, src, AF.Sigmoid)
                elif kind == "vrelu":
                    # relu on DVE: keeps the Q-projection pipeline off the
                    # clogged scalar queue (PE stalls on PSUM reuse otherwise)
                    nc.vector.tensor_scalar_max(dv, src, 0.0)
                else:
                    nc.scalar.activation(dv, src, AF.Relu)

    # K-point phase: chunk (0,512) covers K + KP (+ head of VP)
    emit_proj([(0, 512)])

    # ---- rigid transform helpers -----------------------------------------
    kpco = [pA.tile([128, 6 * 48], F32, name=f"kpco{nt}") for nt in range(NT)]
    vpco = [pA.tile([128, 6 * 96], F32, name=f"vpco{nt}") for nt in range(NT)]
    qco = [pA.tile([128, 6 * 48], F32, name=f"qco{nt}") for nt in range(NT)]

    def _transform(nt, src, dst, bw):
        """Rigid transform of one comp-major tile: nt0 on vector (AP-scalar
        ops), nt1 dirs on gpsimd (broadcast views) so the two row-tiles
        spread across both engines."""
        rt, tr = rot_sb[nt], trans_sb[nt]

        def pv(j):
            return src[nt][:, j * bw:(j + 1) * bw]
        for i in range(3):
            dco = dst[nt][:, i * bw:(i + 1) * bw]
            ddi = dst[nt][:, (3 + i) * bw:(4 + i) * bw]
            if nt == 0:
                nc.vector.tensor_scalar(dco, pv(0), rt[:, 3 * i:3 * i + 1],
                                        tr[:, i:i + 1], OP.mult, OP.add)
                nc.vector.scalar_tensor_tensor(dco, pv(1),
                                               rt[:, 3 * i + 1:3 * i + 2], dco,
                                               OP.mult, OP.add)
                nc.vector.scalar_tensor_tensor(dco, pv(2),
                                               rt[:, 3 * i + 2:3 * i + 3], dco,
                                               OP.mult, OP.add)
                nc.vector.tensor_scalar_mul(ddi, pv(3), rt[:, 3 * i:3 * i + 1])
                nc.vector.scalar_tensor_tensor(ddi, pv(4),
                                               rt[:, 3 * i + 1:3 * i + 2], ddi,
                                               OP.mult, OP.add)
                nc.vector.scalar_tensor_tensor(ddi, pv(5),
                                               rt[:, 3 * i + 2:3 * i + 3], ddi,
                                               OP.mult, OP.add)
            else:
                tdi = tmp_pool.tile([128, bw], F32, tag=f"tdi{bw}", name="tdi")
                nc.vector.tensor_scalar(dco, pv(0), rt[:, 3 * i:3 * i + 1],
                                        tr[:, i:i + 1], OP.mult, OP.add)
                nc.vector.scalar_tensor_tensor(dco, pv(1),
                                               rt[:, 3 * i + 1:3 * i + 2], dco,
                                               OP.mult, OP.add)
                nc.vector.scalar_tensor_tensor(dco, pv(2),
                                               rt[:, 3 * i + 2:3 * i + 3], dco,
                                               OP.mult, OP.add)
                nc.gpsimd.tensor_tensor(
                    ddi, pv(3), rt[:, 3 * i:3 * i + 1].broadcast_to([128, bw]),
                    OP.mult)
                nc.gpsimd.tensor_tensor(
                    tdi[:], pv(4),
                    rt[:, 3 * i + 1:3 * i + 2].broadcast_to([128, bw]), OP.mult)
                nc.gpsimd.tensor_tensor(ddi, ddi, tdi[:], OP.add)
                nc.gpsimd.tensor_tensor(
                    tdi[:], pv(5),
                    rt[:, 3 * i + 2:3 * i + 3].broadcast_to([128, bw]), OP.mult)
                nc.gpsimd.tensor_tensor(ddi, ddi, tdi[:], OP.add)

    def kpcomp(nt, j):  # [128, H, 4] view of K-point comp j
        blk = kpco[nt][:, j * 48:(j + 1) * 48]
        return blk.rearrange("p (h x) -> p h x", x=P)

    def vpcomp(nt, j):  # [128, H, 8] view of V-point comp j
        blk = vpco[nt][:, j * 96:(j + 1) * 96]
        return blk.rearrange("p (h x) -> p h x", x=V)

    def qcomp(nt, j):  # [128, H, 4] view of Q comp block j
        blk = qco[nt][:, j * 48:(j + 1) * 48]
        return blk.rearrange("p (h x) -> p h x", x=QPTS)

    def q_transform(nt):
        _transform(nt, qp_sb, qco, 48)

    # K-point transform immediately after chunk 1
    for nt in range(NT):
        _transform(nt, kp_sb, kpco, 48)

    # remaining K-side projections (VP tail + V) keep the PE busy meanwhile
    emit_proj([(512, 1024), (1024, 1248)])

    # ---- k2 + Kfeat assembly ---------------------------------------------
    k2c = [work.tile([128, H], F32, name=f"k2c{nt}") for nt in range(NT)]

    def psum4(eng, dst, srct):  # [128,48]=(H,4) -> [128,H]
        sv = srct[:].rearrange("p (h x) -> p h x", x=P)
        eng.tensor_tensor(dst, sv[:, :, 0], sv[:, :, 1], OP.add)
        eng.tensor_tensor(dst, dst, sv[:, :, 2], OP.add)
        eng.tensor_tensor(dst, dst, sv[:, :, 3], OP.add)

    kf = [work.tile([128, FEAT * H], BF16, name=f"kf{nt}") for nt in range(NT)]
    kfstgE = work.tile([128, KFC], BF16, name="kfstgE")
    kfstgO = work.tile([128, KFC], BF16, name="kfstgO")
    vastg = work.tile([128, VAC], F8, name="vastg")
    for nt in range(NT):
        eng = nc.gpsimd if nt else nc.vector
        sq = tmp_pool.tile([128, 48], F32, tag=f"sq{nt}k", name="sq")
        t2 = tmp_pool.tile([128, 48], F32, tag=f"t2{nt}k", name="t2")
        eng.tensor_tensor(sq[:], kpcomp(nt, 0), kpcomp(nt, 0), OP.mult)
        for cc in (1, 2):
            eng.tensor_tensor(t2[:], kpcomp(nt, cc), kpcomp(nt, cc), OP.mult)
            eng.tensor_tensor(sq[:], sq[:], t2[:], OP.add)
        psum4(eng, k2c[nt][:], sq)
        nc.vector.tensor_scalar_mul(k2c[nt][:], k2c[nt][:], -gw0 / P)

        kfv = kf[nt][:].rearrange("p (h f) -> p h f", f=FEAT)
        nc.gpsimd.memset(kfv[:, :, 42:64], 0.0)   # pad read by kf transposes
        nc.vector.tensor_copy(kfv[:, :, 0:16],
                              k_sb[nt][:].rearrange("p (h c) -> p h c", c=16))
        for i in range(3):
            # comp-major blocks: coords at 16+4i, dirs at 28+4i (contiguous)
            nc.vector.tensor_copy(kfv[:, :, 16 + 4 * i:20 + 4 * i],
                                  kpcomp(nt, i))
            nc.scalar.copy(kfv[:, :, 28 + 4 * i:32 + 4 * i],
                           kpcomp(nt, 3 + i))
        nc.gpsimd.tensor_copy(kfv[:, :, 40], k2c[nt][:])
        nc.gpsimd.memset(kfv[:, :, 41], 1.0)

    # ---- kfT transposes (84 staged rows) + DMA + kf collective -----------
    for t in range(6):
        for nt in range(NT):
            ps = tpsum.tile([128, 128], BF16, tag="tpsb")
            nc.tensor.transpose(ps[:], kf[nt][:, t * 128:(t + 1) * 128],
                                ident_b[:])
            c0 = t * NB + nt * 128
            eng = nc.scalar if t % 2 else nc.vector
            if t % 2:
                nc.scalar.copy(kfstgE[0:FS, c0:c0 + 128], ps[0:FS, :])
                nc.scalar.copy(kfstgO[0:FS, c0:c0 + 128], ps[FEAT:FEAT + FS, :])
            else:
                nc.vector.tensor_copy(kfstgE[0:FS, c0:c0 + 128], ps[0:FS, :])
                nc.vector.tensor_copy(kfstgO[0:FS, c0:c0 + 128],
                                      ps[FEAT:FEAT + FS, :])
    nc.sync.dma_start(kf_loc[0:FS, :], kfstgE[0:FS, :])
    nc.sync.dma_start(kf_loc[FS:KROWS, :], kfstgO[0:FS, :])
    # trigger kf right away: nothing else sits ahead of it on gpsimd
    nc.gpsimd.collective_compute(
        "AllGather", OP.bypass, replica_groups=GROUPS,
        ins=[kf_loc[:]], outs=[kf_gath[:]])

    # ---- V-point transform + Vall assembly + va collectives --------------
    for nt in range(NT):
        _transform(nt, vp_sb, vpco, 96)
    for nt in range(NT):
        vav = vastg[:].rearrange("p (h y) -> p h y", y=VAC_H)[
            :, :, nt * OCH:(nt + 1) * OCH]
        nc.scalar.copy(vav[:, :, 0:16],
                       v_sb[nt][:].rearrange("p (h c) -> p h c", c=16))
        for j in range(6):
            eng = nc.vector if nt == 0 else nc.scalar
            if nt == 0:
                nc.vector.tensor_copy(vav[:, :, 16 + j * V:16 + (j + 1) * V],
                                      vpcomp(nt, j))
            else:
                nc.scalar.copy(vav[:, :, 16 + j * V:16 + (j + 1) * V],
                               vpcomp(nt, j))
        nc.gpsimd.memset(vav[:, :, 64], 1.0)
        nc.gpsimd.memset(vav[:, :, 65], 0.0)
    nc.sync.dma_start(va_loc[:, :], vastg[:, :])
    nc.gpsimd.collective_compute(
        "AllGather", OP.bypass, replica_groups=GROUPS,
        ins=[va_loc[:]], outs=[va_gath[:]])

    # ---- Q-phase projections (overlap the collectives) -------------------
    emit_proj([(1248, 1648), (1648, WALL_COLS)])

    # ---- qscale on the scalar queue (needed by qf mid-Q-side) ------------
    qscale_sb = const.tile([128, FEAT * H], F32)
    nc.scalar.dma_start(qscale_sb[:], qscale_d[:, :])

    # ---- Q side (overlaps collectives) -----------------------------------
    # dep-gate: keep gpsimd's in-order queue clear until the kf staging DMA
    # fires so the collective triggers are not stuck behind Q-side gpsimd
    # work (the [0,0] garbage is overwritten by the transform below).
    nc.gpsimd.tensor_copy(qco[1][0:1, 0:1], kfstgO[0:1, 0:1])
    for nt in range(NT):
        q_transform(nt)
        gv = g_sb[nt][:].rearrange("p (h x) -> p h x", x=P)
        for i in range(6):
            eng = nc.vector if i < 3 else nc.gpsimd
            eng.tensor_tensor(qcomp(nt, i), qcomp(nt, i), gv, OP.mult)

    # NOTE: the reference's q2 and curvature terms are constant over the
    # softmax (key) axis, so they cancel in the normalization and are not
    # computed at all; qf col 41 is zeroed instead (kf col 41 is ones).

    qf = [work.tile([128, FEAT * H], BF16, name=f"qf{nt}") for nt in range(NT)]
    for nt in range(NT):
        qfv = qf[nt][:].rearrange("p (h f) -> p h f", f=FEAT)
        nc.vector.tensor_copy(qfv[:, :, 0:16],
                              q_sb[nt][:].rearrange("p (h c) -> p h c", c=16))
        for i in range(3):
            nc.vector.tensor_copy(qfv[:, :, 16 + 4 * i:20 + 4 * i],
                                  qcomp(nt, i))
            nc.vector.tensor_copy(qfv[:, :, 28 + 4 * i:32 + 4 * i],
                                  qcomp(nt, 3 + i))
        nc.gpsimd.memset(qfv[:, :, 40], 1.0)
        nc.gpsimd.memset(qfv[:, :, 41], 0.0)
        nc.vector.tensor_tensor(qf[nt][:], qf[nt][:], qscale_sb[:], OP.mult)

    # block-diagonal paired-query layout: cols 0:256 even head (rows 0:42),
    # cols 256:512 odd head (rows 42:84); everything else zero.  Engine ops
    # cannot write at partition offset 42, so the odd halves hop through a
    # staging tile and land via SBUF->SBUF DMA (no offset restriction).
    qfT2 = [work.tile([KROWS, 2 * NB], BF16, name=f"qfT{t}") for t in range(6)]
    qto = work.tile([128, 6 * NB], BF16, name="qto")
    for t in range(6):
        nc.gpsimd.memset(qfT2[t][:], 0.0)
        for nt in range(NT):
            ps = tpsum.tile([128, 128], BF16, tag="tpsb")
            nc.tensor.transpose(ps[:], qf[nt][:, t * 128:(t + 1) * 128],
                                ident_b[:])
            eng = nc.scalar if t % 2 else nc.vector
            if t % 2:
                nc.scalar.copy(qfT2[t][0:FS, nt * 128:(nt + 1) * 128],
                               ps[0:FS, :])
                nc.scalar.copy(qto[0:FS, t * NB + nt * 128:t * NB + (nt + 1) * 128],
                               ps[FEAT:FEAT + FS, :])
            else:
                nc.vector.tensor_copy(qfT2[t][0:FS, nt * 128:(nt + 1) * 128],
                                      ps[0:FS, :])
                nc.vector.tensor_copy(
                    qto[0:FS, t * NB + nt * 128:t * NB + (nt + 1) * 128],
                    ps[FEAT:FEAT + FS, :])
    for t in range(6):
        nc.scalar.dma_start(qfT2[t][FS:KROWS, NB:2 * NB],
                            qto[0:FS, t * NB:(t + 1) * NB])

    # ---- gather-in (coalesced 3D DMAs, pipelined with attention) ---------
    pA_ctx.close()
    attA = ctx.enter_context(tc.tile_pool(name="attA", bufs=1))
    # kfAllT col = t*1024 + g*256 + half*128; rows 0:42 even head, 42:84 odd
    # (DMA writes have no partition-offset restriction)
    kfAllT = attA.tile([KROWS, 6 * 1024], BF16, name="kfAllT")
    # vaAll per-g block [128, VAC], head-major inside; the fp8 wire copy
    # lands in vaAll8 and is upcast to bf16 for the AV matmuls
    vaAll = attA.tile([128, 4 * VAC], BF16, name="vaAll")
    vaAll8 = attA.tile([128, 4 * VAC], F8, name="vaAll8")
    for t in range(6):
        dst = kfAllT[:, t * 1024:(t + 1) * 1024].rearrange(
            "p (g c) -> p g c", g=4)
        nc.sync.dma_start(dst, kf_gath[:, :, t * NB:(t + 1) * NB].rearrange(
            "g p c -> p g c"))
    dstva = vaAll8[:].rearrange("p (g c) -> p g c", c=VAC)
    nc.sync.dma_start(dstva, va_gath[:, :, :].rearrange("g p c -> p g c"))
    for g in range(4):
        cv = slice(g * VAC, (g + 1) * VAC)
        nc.vector.tensor_copy(vaAll[:, cv], vaAll8[:, cv])



    # ---- attention -------------------------------------------------------
    pre_ctx.close()
    att_ctx = ExitStack()
    apsum = att_ctx.enter_context(tc.tile_pool(name="apsum", bufs=2, space=PS))
    opsum = att_ctx.enter_context(tc.tile_pool(name="opsum", bufs=2, space=PS))
    # expT layout per head-pair t: col = kb*512 + parity*256 + q
    expT_tiles = [attA.tile([128, 4096], BF16, name=f"expT{i}")
                  for i in range(6)]
    o_all = [work.tile([128, FEAT * H], F32, name=f"oall{qt}") for qt in range(NT)]
    sums = [work.tile([128, H], F32, name=f"sums{qt}") for qt in range(NT)]

    # all QK+exp first (only needs kf); AV chases the va collectives
    for t in range(6):
        expT = expT_tiles[t]
        for pair in range(4):
            aps = apsum.tile([128, 1024], F32, tag="attT", name="aps")
            for half in range(2):
                kb = 2 * pair + half
                nc.tensor.matmul(
                    aps[:, half * 512:(half + 1) * 512],
                    kfAllT[:, t * 1024 + kb * 128:t * 1024 + (kb + 1) * 128],
                    qfT2[t][:, :],
                    start=True, stop=True)
            nc.scalar.activation(
                expT[:, pair * 1024:(pair + 1) * 1024], aps[:], AF.Exp)

    # deferred output-proj weights.  The 1-elem copies from qfT2 are
    # artificial dependencies: without them the scheduler starts these
    # 1.3MB loads (and the fp32r casts) at t=0, stealing front DMA/gpsimd
    # time from the collective-staging critical path.  Plain F32 DMA: an
    # F32R-dtyped DMA poisons concurrent bf16 transfers on shared SDMA
    # engines (tf32 rounding of word pairs); the fp32r rounding happens in
    # an on-chip copy that rides gpsimd while the QK/exp stream runs.
    wout_f, wout_sb = [], []
    for kc in range(KCH):
        r0 = kc * 128
        r1 = min(FOUT + 2, r0 + 128)
        t = const.tile([r1 - r0, CS], F32, name=f"woutf{kc}")
        nc.gpsimd.tensor_copy(t[0:1, 0:1], qfT2[0][0:1, 0:1])
        nc.sync.dma_start(t[:], wout_d[r0:r1, :])
        wout_f.append(t)
        wout_sb.append(const.tile([r1 - r0, CS], F32R, name=f"wout{kc}"))
    for kc in range(KCH):
        nc.gpsimd.tensor_copy(wout_sb[kc][:], wout_f[kc][:])

    # ---- inverse transform helper (per qt, per 6-head half) --------------
    feats = [work.tile([128, FOUT], F32, name=f"feats{qt}") for qt in range(NT)]

    def emit_inverse(qt, h0, nh):
        # qt 0 runs on vector (AP-scalar ops), qt 1 on gpsimd (broadcast
        # views) so the two row-tiles proceed concurrently.
        hs = slice(h0, h0 + nh)
        W = nh * V
        ovv = o_all[qt][:].rearrange("p (h f) -> p h f", f=FEAT)[:, hs]

        def og(j):  # [128, nh, V] component j (component-major o layout)
            return ovv[:, :, 16 + j * V:16 + (j + 1) * V]

        gview = feats[qt][:, 192:FOUT].rearrange(
            "p (h x c) -> p h x c", h=H, c=7)[:, hs]

        def gcol(i):  # [128, nh*V] flat strided view of feats column i
            return gview[:, :, :, i].rearrange("p h x -> p (h x)")

        rt, tr = rot_sb[qt], trans_sb[qt]
        shp = [128, nh, V]
        eng = nc.vector if qt == 0 else nc.gpsimd

        # fold the softmax normalization in here: o_scalar and coord points
        # are divided by the exp-sum; direction points skip it (they get
        # re-normalized anyway).
        rec6 = tmp_pool.tile([128, nh], F32, tag=f"rec6{qt}{nh}", name="rec6")
        nc.vector.reciprocal(rec6[:], sums[qt][:, hs])
        fview = feats[qt][:, h0 * 16:(h0 + nh) * 16].rearrange(
            "p (h c) -> p h c", c=16)
        eng.tensor_tensor(fview, ovv[:, :, 0:16],
                          rec6[:].unsqueeze(2).broadcast_to([128, nh, 16]),
                          OP.mult)

        ogs = [tmp_pool.tile([128, W], F32, tag=f"ogs{j}{qt}{nh}",
                             name=f"ogs{j}") for j in range(3)]
        ld = [tmp_pool.tile([128, W], F32, tag=f"ld{i}{qt}{nh}",
                            name=f"ld{i}") for i in range(3)]
        n2 = tmp_pool.tile([128, W], F32, tag=f"n2{qt}{nh}", name="n2")
        t2b = tmp_pool.tile([128, W], F32, tag=f"t2b{qt}{nh}", name="t2b")

        def v3(t_):
            return t_[:].rearrange("p (h x) -> p h x", x=V)

        r6b = rec6[:].unsqueeze(2).broadcast_to(shp)
        for j in range(3):
            eng.tensor_tensor(v3(ogs[j]), og(j), r6b, OP.mult)
        if qt == 0:
            for j in range(3):
                eng.tensor_scalar(ogs[j][:], ogs[j][:], tr[:, j:j + 1], None,
                                  OP.subtract)
            for i in range(3):
                # lc accumulates directly into the feats column
                eng.tensor_scalar_mul(gcol(i), ogs[0][:], rt[:, i:i + 1])
                eng.scalar_tensor_tensor(gcol(i), ogs[1][:], rt[:, 3 + i:4 + i],
                                         gcol(i), OP.mult, OP.add)
                eng.scalar_tensor_tensor(gcol(i), ogs[2][:], rt[:, 6 + i:7 + i],
                                         gcol(i), OP.mult, OP.add)
                eng.tensor_scalar_mul(v3(ld[i]), og(3), rt[:, i:i + 1])
                eng.scalar_tensor_tensor(v3(ld[i]), og(4), rt[:, 3 + i:4 + i],
                                         v3(ld[i]), OP.mult, OP.add)
                eng.scalar_tensor_tensor(v3(ld[i]), og(5), rt[:, 6 + i:7 + i],
                                         v3(ld[i]), OP.mult, OP.add)
        else:
            # lc path stays on gpsimd; ld path goes to vector (AP-scalar ops)
            # so the two row-tiles' tail work spreads across both engines.
            tmpg = tmp_pool.tile([128, W], F32, tag=f"tmpg{qt}{nh}",
                                 name="tmpg")
            for j in range(3):
                eng.tensor_tensor(ogs[j][:], ogs[j][:],
                                  tr[:, j:j + 1].broadcast_to([128, W]),
                                  OP.subtract)
            for i in range(3):
                gc = gview[:, :, :, i]
                eng.tensor_tensor(gc, v3(ogs[0]),
                                  rt[:, i:i + 1].broadcast_to(shp), OP.mult)
                eng.tensor_tensor(v3(tmpg), v3(ogs[1]),
                                  rt[:, 3 + i:4 + i].broadcast_to(shp), OP.mult)
                eng.tensor_tensor(gc, gc, v3(tmpg), OP.add)
                eng.tensor_tensor(v3(tmpg), v3(ogs[2]),
                                  rt[:, 6 + i:7 + i].broadcast_to(shp), OP.mult)
                eng.tensor_tensor(gc, gc, v3(tmpg), OP.add)
                nc.vector.tensor_scalar_mul(v3(ld[i]), og(3), rt[:, i:i + 1])
                nc.vector.scalar_tensor_tensor(v3(ld[i]), og(4),
                                               rt[:, 3 + i:4 + i], v3(ld[i]),
                                               OP.mult, OP.add)
                nc.vector.scalar_tensor_tensor(v3(ld[i]), og(5),
                                               rt[:, 6 + i:7 + i], v3(ld[i]),
                                               OP.mult, OP.add)

        dng = nc.gpsimd if qt == 0 else nc.vector   # ld-norm engine (opposite)
        eng.tensor_tensor(n2[:], gcol(0), gcol(0), OP.mult)
        for i in (1, 2):
            eng.tensor_tensor(t2b[:], gcol(i), gcol(i), OP.mult)
            eng.tensor_tensor(n2[:], n2[:], t2b[:], OP.add)
        nc.scalar.activation(gcol(6), n2[:], AF.Sqrt)
        n2d = tmp_pool.tile([128, W], F32, tag=f"n2d{qt}{nh}", name="n2d")
        t2d = tmp_pool.tile([128, W], F32, tag=f"t2d{qt}{nh}", name="t2d")
        dng.tensor_tensor(n2d[:], ld[0][:], ld[0][:], OP.mult)
        for i in (1, 2):
            dng.tensor_tensor(t2d[:], ld[i][:], ld[i][:], OP.mult)
            dng.tensor_tensor(n2d[:], n2d[:], t2d[:], OP.add)
        nc.scalar.activation(n2d[:], n2d[:], AF.Sqrt)
        nc.vector.tensor_scalar_max(n2d[:], n2d[:], EPS)
        nc.vector.reciprocal(n2d[:], n2d[:])
        for i in range(3):
            dng.tensor_tensor(gcol(3 + i), ld[i][:], n2d[:], OP.mult)

    def emit_av_mm(h):
        expT = expT_tiles[h // 2]
        par = (h % 2) * NB
        ot_ps = opsum.tile([OCH, NB], F32, tag="otacc", name="ot_ps")
        for kb in range(NKB):
            voff = (kb // 2) * VAC + h * VAC_H + (kb % 2) * OCH
            ecol = kb * 512 + par
            nc.tensor.matmul(
                ot_ps[:],
                vaAll[:, voff:voff + OCH],
                expT[:, ecol:ecol + NB],
                start=(kb == 0), stop=(kb == NKB - 1))
        # early drains stay off the scalar queue so the exp stream never
        # waits behind them; late heads drain via scalar (exps done) so the
        # vector queue is free for the interleaved inverse.
        ot_sb = tmp_pool.tile([OCH, NB], F32R, tag="otsb", name="otsb", bufs=3)
        if h >= 8:
            nc.scalar.copy(ot_sb[:], ot_ps[:])
        else:
            nc.vector.tensor_copy(ot_sb[:], ot_ps[:])
        return ot_sb

    def emit_av_out(h, ot_sb):
        for qt in range(NT):
            tp = opsum.tile([128, OCH], F32R, tag="otp", name="tp")
            nc.tensor.transpose(tp[:], ot_sb[:, qt * 128:(qt + 1) * 128],
                                ident_r[0:OCH, 0:OCH])
            # unnormalized output + exp-sum; division happens in the inverse.
            # qt0 lands via vector, qt1 via scalar to balance the two queues.
            if qt == 0:
                nc.vector.tensor_copy(o_all[qt][:, h * FEAT:h * FEAT + 64],
                                      tp[:, 0:64].bitcast(F32))
                nc.vector.tensor_copy(sums[qt][:, h:h + 1],
                                      tp[:, 64:65].bitcast(F32))
            else:
                nc.scalar.copy(o_all[qt][:, h * FEAT:h * FEAT + 64],
                               tp[:, 0:64].bitcast(F32))
                nc.scalar.copy(sums[qt][:, h:h + 1], tp[:, 64:65].bitcast(F32))

    # AV output transposes are deferred by one head so the PE never waits on
    # the vector drain of ot_sb; the inverse for heads 0-5 fires as soon as
    # head 5's outputs land.
    pend = []
    for h in range(H):
        while pend:
            hp, sb = pend.pop(0)
            emit_av_out(hp, sb)
            if hp == 5:
                for qt in range(NT):
                    emit_inverse(qt, 0, 6)
            elif hp == 9:
                for qt in range(NT):
                    emit_inverse(qt, 6, 4)
        pend.append((h, emit_av_mm(h)))
    while pend:
        hp, sb = pend.pop(0)
        emit_av_out(hp, sb)

    # ---- tail: last head-pair inverse + out-proj -------------------------
    att_ctx.close()
    tpsum2 = ctx.enter_context(tc.tile_pool(name="tpsum2", bufs=2, space=PS))
    opsum2 = ctx.enter_context(tc.tile_pool(name="opsum2", bufs=2, space=PS))
    for qt in range(NT):
        emit_inverse(qt, 10, 2)

    fT = []
    for kc in range(KCH):
        r0 = kc * 128
        rw = min(FOUT, r0 + 128) - r0          # 128 or 96
        pw = rw + 2 if kc == KCH - 1 else rw   # +2 ones rows on last chunk
        t = work.tile([pw, NB], F32R, name=f"fT{kc}")
        fT.append(t)
    lastr = FOUT - (KCH - 1) * 128
    nc.vector.tensor_copy(fT[KCH - 1][lastr:lastr + 2, :], ones2_f32[:])
    ps_out = [opsum2.tile([128, CS], F32, tag=f"oproj{qt}", name="ps")
              for qt in range(NT)]
    # kc 0..3 read only hh=0 regions of feats (+ the o_scalar copies done
    # first thing in emit_inverse) so their transposes/matmuls run on PE
    # while the hh=1 inverse is still on vector/gpsimd.
    for kc in range(KCH):
        r0 = kc * 128
        rw = min(FOUT, r0 + 128) - r0
        for qt in range(NT):
            ps = tpsum2.tile([128, 128], F32, tag="tps2")
            nc.tensor.transpose(ps[:rw, :], feats[qt][:, r0:r0 + rw], ident[:])
            if (kc + qt) % 2:
                nc.vector.tensor_copy(fT[kc][:rw, qt * 128:(qt + 1) * 128],
                                      ps[:rw, :])
            else:
                nc.scalar.copy(fT[kc][:rw, qt * 128:(qt + 1) * 128], ps[:rw, :])
        for qt in range(NT):
            nc.tensor.matmul(ps_out[qt][:], fT[kc][:, qt * 128:(qt + 1) * 128],
                             wout_sb[kc][:], start=(kc == 0),
                             stop=(kc == KCH - 1))
    for qt in range(NT):
        osb = tmp_pool.tile([128, CS], F32, tag="osb", name="osb")
        nc.scalar.copy(osb[:], ps_out[qt][:])
        nc.sync.dma_start(out_loc[qt * 128:(qt + 1) * 128, :], osb[:])

    if _DEBUG_DUMP:
        dbg_kfg = nc.dram_tensor("dbg_kfg", [4, KROWS, KFC], BF16,
                                 kind="ExternalOutput")

        dbg_oall = nc.dram_tensor("dbg_oall", [NT, 128, FEAT * H], F32,
                                  kind="ExternalOutput")
        dbg_sums = nc.dram_tensor("dbg_sums", [NT, 128, H], F32,
                                  kind="ExternalOutput")
        nc.sync.dma_start(dbg_kfg[:], kf_gath[:])
        pass  # debug va dump removed
        for qt in range(NT):
            nc.sync.dma_start(dbg_oall[qt], o_all[qt][:])
            nc.sync.dma_start(dbg_sums[qt], sums[qt][:])


def _run(inputs, trace=False):
    s, rot9, trans, wall, wout_b, qscale, gw = _host_prep(inputs)
    nc = _build_program(float(gw[0]), float(gw[1]))
    in_maps = []
    for c in range(8):
        b, qb = c // 4, c % 4
        r = slice(qb * NB, (qb + 1) * NB)
        in_maps.append({
            "s_loc": np.ascontiguousarray(s[b, r]),
            "rot_loc": np.ascontiguousarray(rot9[b, r]),
            "trans_loc": np.ascontiguousarray(trans[b, r]),
            "wall": wall, "wout_b": wout_b, "qscale": qscale,
        })
    res = run_bass_kernel_spmd(nc, in_maps, list(range(8)), trace=trace)
    out = np.empty((B, N, CS), np.float32)
    for c in range(8):
        b, qb = c // 4, c % 4
        out[b, qb * NB:(qb + 1) * NB] = res.results[c]["out_loc"]
    return out, res


_LAST_RES = None


def kernel(**inputs):
    global _LAST_RES
    out, _LAST_RES = _run(inputs, trace=False)
    return out


def kernel_traced(**inputs):
    return _run(inputs, trace=True)
